# revision 54
# baseline (speedup 1.0000x reference)
"""Trainium2 Bass kernel for the GNN message-passing module.

Per-sample pipeline (data-parallel: one batch element per NeuronCore):
  1. segment sums via one-hot matmul on PE (x^T arrives pre-transposed
     and pre-packed from the host; one accumulating matmul per 128-px
     chunk keeps the HAM clock gate at K=8/8),
  2. small "middle" stage: means, M=W@W^T, Mahalanobis adjacency folded
     into a (K, C_out) table; group-0 conv matmuls are prefilled across
     the middle's serial chain to keep the PE busy,
  3. out = conv_w @ x + table2T[index] via PE matmuls (the gather is a
     one-hot matmul accumulated into the same PSUM as the 1x1 conv);
     G=4 tiles share each stationary weight load (LDWEIGHTS is not
     hidden between distinct-weight matmuls), one output DMA per group
     in a packed layout the host descrambles. A PSUM bank must never
     hold two open accumulation groups (the h=0/h=1 halves of a po tile
     share a bank, so each h's group fully closes before the next).

Math notes:
  adj[i,j] = exp(-(m_j-m_i)^T M (m_j-m_i)) with zero diagonal, M=W W^T.
  Using G = means @ M @ means^T, g = diag(G):
    adj[i,j] = exp(2G_ij - g_i - g_j) - delta_ij
  agg = adj @ means  =>  out += conv_w @ agg[index]
  table2T[k,:] = e^{-g_k} * (aggT_raw^T @ conv_w^T)[k,:] - (means @ conv_w^T)[k,:]
  where aggT_raw[:,i] = sum_j B[j,i] * (e^{-g_j} means[j,:]),
        B[i,j] = exp(2G_ij - g_i).

Precision: single bf16 plane for x / conv_w / table / output with fp32
PSUM accumulation; middle stage fp32. Whole-pipeline numpy sim gives
rel err ~4e-3 against the fp32 reference (gate 2e-2). Segment counts
(and their reciprocals) are computed on the host and passed in.
"""

import os
import sys

import numpy as np


def _ensure_path():
    try:
        import concourse  # noqa: F401
    except ImportError:
        for p in ("/opt/trn_rl_repo", os.path.expanduser("~/.axon_site/_ro/trn_rl_repo")):
            if os.path.isdir(p) and p not in sys.path:
                sys.path.insert(0, p)


_ensure_path()
# persistent jax/XLA executable cache: makes repeat compiles of the same
# kernel cheap across processes (first compile of a variant is ~minutes).
os.environ.setdefault("JAX_COMPILATION_CACHE_DIR", "/tmp/jax_neff_cache")
os.environ.setdefault("JAX_PERSISTENT_CACHE_MIN_COMPILE_TIME_SECS", "10")

import concourse.bass as bass  # noqa: E402
import concourse.tile as tile  # noqa: E402
from concourse import bacc  # noqa: E402
from concourse import mybir  # noqa: E402
from concourse.masks import make_identity  # noqa: E402

F32 = mybir.dt.float32

# --- workaround: this walrus build rejects instructions carrying >2 sem
# waits ("Too many sync wait commands" in setupSyncWait). TileContext's exit
# drain accumulates one wait per outstanding processor (DMA queues etc.), so
# split them across NOPs emitted just before the drain. Semaphores are
# monotonic, so waiting earlier on the same conditions is equivalent.
_MAX_WAITS = 1
_drain_patched = False


def _patch_tile_drain():
    global _drain_patched
    if _drain_patched:
        return
    _drain_patched = True
    from concourse.vector_clock import ScopedClock

    orig = tile.TileContext._drain_and_barrier

    def patched(self, tick_clock, wait_clock):
        nc = self.nc
        probe = nc.sync.nop()
        wait_clock.add_sem_waits(
            probe.ins, ScopedClock({None: tick_clock.global_clock})
        )
        waits = list(probe.ins.sync_info.on_wait or [])
        chunks = [waits[i:i + _MAX_WAITS] for i in range(0, len(waits), _MAX_WAITS)]
        probe.ins.sync_info.on_wait = chunks[0] if chunks else []
        for chunk in chunks[1:]:
            nop = nc.sync.nop()
            nop.ins.sync_info = mybir.SyncInfo(on_wait=chunk, on_update=[])
        orig(self, tick_clock, wait_clock)
        _trim_redundant_waits(nc)

    tile.TileContext._drain_and_barrier = patched


def _trim_redundant_waits(nc):
    """Transitive wait reduction. Tile's add_semaphores is per-instruction
    minimal but not transitively minimal across processors: an instruction
    often carries waits already implied by (a) an earlier wait on the same
    engine, or (b) the closure of another wait it carries (the producer's own
    waits + in-order retirement on the producer's engine). This walrus build
    rejects instructions with >2 sync waits, so prune implied waits.

    Soundness assumptions: sem updates fire at instruction retirement;
    retirement is in-order per compute engine and per DMA queue sem (one sem
    per queue, FIFO); a kept wait on sem S>=v implies the v-reaching update's
    instruction retired, hence its dispatch-time holds and (non-DMA) all
    earlier same-engine updates.
    """
    import bisect

    for blk in nc.m.functions[0].blocks:
        insts = list(blk.instructions)
        n = len(insts)
        # sems that are ever decremented/reset are not monotonic; leave all
        # waits on them untouched and exclude them from closures (barrier
        # gather/release sems, end-of-kernel sem clears).
        nonmono = set()
        for ins in insts:
            si = ins.sync_info
            if si and si.on_update:
                for u in si.on_update:
                    if u.update_mode != "sem-inc":
                        nonmono.add(u.id)
            try:
                if ins.is_reset_sema:
                    lo = ins.reset_range_start
                    hi = ins.reset_range_stop
                    if lo is not None and hi is not None:
                        nonmono.update(range(lo, hi + 1))
            except Exception:
                pass
        upd = {}
        cum = {}
        own_cum_after = [None] * n
        eng_of = [str(i.engine) for i in insts]
        is_dma = [type(i).__name__ == "InstDMACopy" for i in insts]
        for idx, ins in enumerate(insts):
            si = ins.sync_info
            d = {}
            if si and si.on_update:
                for u in si.on_update:
                    if (u.update_mode != "sem-inc" or not u.update_value
                            or u.id in nonmono):
                        continue
                    c = cum.get(u.id, 0) + u.update_value
                    cum[u.id] = c
                    upd.setdefault(u.id, []).append((c, idx))
                    d[u.id] = c
            own_cum_after[idx] = d
        eng_cum_after = [None] * n
        run = {}
        for idx in range(n):
            e = eng_of[idx]
            m = dict(run.get(e, {}))
            if not is_dma[idx]:
                for s, c in own_cum_after[idx].items():
                    m[s] = c
            run[e] = m
            eng_cum_after[idx] = m

        def updater_idx(sem, v):
            lst = upd.get(sem)
            if not lst:
                return None
            pos = bisect.bisect_left(lst, (v, -1))
            if pos == len(lst):
                return None
            return lst[pos][1]

        holds_at = [None] * n
        last_eng = {}
        memo = {}

        def completion_holds(uidx):
            if uidx in memo:
                return memo[uidx]
            h = dict(holds_at[uidx] or {})
            src_cum = own_cum_after[uidx] if is_dma[uidx] else eng_cum_after[uidx]
            for s, c in src_cum.items():
                if h.get(s, 0) < c:
                    h[s] = c
            memo[uidx] = h
            return h

        n_dropped = 0
        for idx, ins in enumerate(insts):
            e = eng_of[idx]
            base = dict(holds_at[last_eng[e]]) if e in last_eng else {}
            si = ins.sync_info
            if si and si.on_wait:
                kept = []
                for w in si.on_wait:
                    if w.wait_mode != "sem-ge-imm" or w.id in nonmono:
                        kept.append(w)
                        continue
                    if base.get(w.id, 0) >= w.wait_value:
                        n_dropped += 1
                        continue
                    kept.append(w)
                    ui = updater_idx(w.id, w.wait_value)
                    if ui is not None and ui < idx:
                        for s, v in completion_holds(ui).items():
                            if base.get(s, 0) < v:
                                base[s] = v
                    if base.get(w.id, 0) < w.wait_value:
                        base[w.id] = w.wait_value
                if len(kept) != len(si.on_wait):
                    si.on_wait = kept
            holds_at[idx] = base
            last_eng[e] = idx
_compile_patched = False


def _patch_compile_bir():
    """This walrus build accepts at most ONE sync wait per instruction in
    several encodings (S3_LW matmuls, CTRL NoOp/Drain). Tile legitimately
    emits 2 waits on some instructions, so rewrite the serialized BIR just
    before walrus: keep one wait on the instruction and hoist the rest onto
    same-engine NoOps inserted immediately before it (same dispatch point,
    so semantics are unchanged)."""
    global _compile_patched
    if _compile_patched:
        return
    _compile_patched = True
    import orjson

    from concourse import bass2jax, bass_utils

    orig = bass_utils.compile_bir_kernel

    def _split_waits(bir_json: bytes) -> bytes:
        d = orjson.loads(bir_json)
        changed = False
        for fn in d.get("functions", []):
            for blk in fn.get("blocks", []):
                insts = blk.get("instructions", [])
                out = []
                for inst in insts:
                    si = inst.get("sync_info") or {}
                    ow = si.get("on_wait") or []
                    if len(ow) > 1:
                        changed = True
                        for k, w in enumerate(ow[:-1]):
                            out.append({
                                "debug": inst.get("debug", 0),
                                "engine": inst["engine"],
                                "ins": [],
                                "name": f"{inst['name']}-w{k}",
                                "opcode": "NoOp",
                                "outs": [],
                                "sync_info": {"on_update": [],
                                              "on_wait": [w]},
                            })
                        si["on_wait"] = [ow[-1]]
                    out.append(inst)
                blk["instructions"] = out
        return orjson.dumps(d) if changed else bir_json

    def wrapper(bir_json, tmpdir, neff_name="file.neff"):
        return orig(_split_waits(bir_json), tmpdir, neff_name=neff_name)

    bass_utils.compile_bir_kernel = wrapper
    bass2jax.compile_bir_kernel = wrapper


AF = mybir.ActivationFunctionType
ALU = mybir.AluOpType

B, C, K, H, W_DIM = 8, 256, 64, 128, 128
HW = H * W_DIM  # 16384 pixels per sample
N_CORES = 8

PX_TILE = 2048        # pass-1 x DMA tile (pixels)
P2_TILE = 256         # pass-2 pixel tile (N=256 keeps the HAM clock-gate at
                      # K=8/8: N=512 streams issue too slowly and get stuck
                      # at half clock — measured 500ns/MM vs ~150ns)


def build_nc():
    _patch_tile_drain()
    _patch_compile_bir()
    # Bacc (not raw Bass): its compile() pass auto-inserts the GPSIMD
    # library reloads that ap_gather needs, in correct program order.
    nc = bacc.Bacc("TRN2", target_bir_lowering=False, debug=False)
    BF16 = mybir.dt.bfloat16
    idx_d = nc.dram_tensor("idxf", (HW,), F32, kind="ExternalInput")
    wt_d = nc.dram_tensor("wt", (C, C), F32, kind="ExternalInput")      # W^T
    # output in pass-2 group-tile order (host descrambles): [g, p, i*512+h*256+w]
    out_d = nc.dram_tensor("out", (HW // (4 * P2_TILE), 128, 4 * 2 * P2_TILE),
                           BF16, kind="ExternalOutput")
    ins = dict(
        xh=nc.dram_tensor("xh", (C, HW), BF16, kind="ExternalInput").ap(),
        # x^T pre-packed on host into per-px-tile SBUF layout: [t, p, u*256+c]
        # so each DMA line is 8KB contiguous per partition
        xth=nc.dram_tensor("xth", (HW // PX_TILE, 128,
                                   (PX_TILE // 128) * C), BF16,
                           kind="ExternalInput").ap(),
        cwth=nc.dram_tensor("cwth", (C, C), BF16, kind="ExternalInput").ap(),
        cwt=nc.dram_tensor("cwt", (C, C), F32, kind="ExternalInput").ap(),
        idxbf=nc.dram_tensor("idxbf", (HW,), BF16, kind="ExternalInput").ap(),
        recip=nc.dram_tensor("recip", (K,), F32, kind="ExternalInput").ap(),
    )

    with tile.TileContext(nc) as tc:
        _body(tc, ins, idx_d.ap(), wt_d.ap(), out_d.ap())
    nc.compile()
    return nc


def _body(tc, ins, idx_v, wt_v, out_v):
    nc = tc.nc
    BF16 = mybir.dt.bfloat16
    n_px_tiles = HW // PX_TILE              # 8
    chunks_per_px_tile = PX_TILE // 128     # 16
    n_p2_tiles = HW // P2_TILE              # 32
    cwt_v = ins["cwt"]

    G = 4                                   # pass-2 tiles per weight-reuse group
    n_groups = n_p2_tiles // G              # 16

    with (
        tc.tile_pool(name="consts", bufs=1) as consts,
        tc.tile_pool(name="xres", bufs=n_px_tiles) as xres,
        tc.tile_pool(name="mid_sb", bufs=1) as mid_sb,
        tc.tile_pool(name="p2_sb", bufs=3) as p2_sb,
    ):
        # ---- constants / parameter loads ----
        ident = consts.tile([128, 128], F32, tag="ident")
        make_identity(nc, ident[:])

        iota_row = consts.tile([128, K], F32, tag="iota_row")  # [p,k] = k
        iota_row_i = consts.tile([128, K], mybir.dt.int32, tag="iota_row_i")
        nc.gpsimd.iota(iota_row_i[:], pattern=[[1, K]], base=0, channel_multiplier=0)
        nc.vector.tensor_copy(iota_row[:], iota_row_i[:])

        negI = consts.tile([K, K], F32, tag="negI")            # -identity(64)
        nc.gpsimd.memset(negI[:], 0.0)
        nc.gpsimd.affine_select(
            out=negI[:], in_=negI[:], compare_op=ALU.not_equal,
            fill=-1.0, base=0, pattern=[[-1, K]], channel_multiplier=1,
        )

        wt_sb = consts.tile([128, 2, C], F32, tag="wt_sb")     # [e, j, c] = W^T[j*128+e, c]
        nc.sync.dma_start(out=wt_sb[:], in_=wt_v.rearrange("(j p) c -> p j c", p=128))
        cwt_sb = consts.tile([128, 2, C], F32, tag="cwt_sb")   # [ci, j, co]
        nc.sync.dma_start(out=cwt_sb[:], in_=cwt_v.rearrange("(j p) c -> p j c", p=128))
        cwth_sb = consts.tile([128, 2, C], BF16, tag="cwth_sb")
        nc.sync.dma_start(
            out=cwth_sb[:],
            in_=ins["cwth"].rearrange("(j p) c -> p j c", p=128))

        idx_pm = consts.tile([128, 128], F32, tag="idx_pm")    # [p,f] = idx[p*128+f]
        nc.sync.dma_start(out=idx_pm[:], in_=idx_v.rearrange("(p f) -> p f", p=128))

        recip_sb = consts.tile([K, 1], F32, tag="recip_sb")    # 1/count (host)
        nc.sync.dma_start(out=recip_sb[:], in_=ins["recip"].rearrange("(p o) -> p o", o=1))

        iota_col = consts.tile([K, 1], F32, tag="iota_col")    # [k,0] = k
        iota_col_i = consts.tile([K, 1], mybir.dt.int32, tag="iota_col_i")
        nc.gpsimd.iota(iota_col_i[:], pattern=[[1, 1]], base=0,
                       channel_multiplier=1)
        nc.vector.tensor_copy(iota_col[:], iota_col_i[:])
        # index values broadcast to K partitions (bf16: 0..63 exact); the
        # DMA is emitted mid-way through the pass-1 xT stream (needed ~20us
        # in, must not delay the first xT tiles).
        idx_bc = consts.tile([K, HW], BF16, tag="idx_bc")
        # pass-2 one-hot [k, px], pre-generated during pass1/middle so the
        # pass-2 PE stream never round-trips through DVE (HAM warmth).
        oh2_all = consts.tile([K, HW], BF16, tag="oh2_all")

        M_sb = mid_sb.tile([128, 2, C], F32, tag="M_sb")       # M = W @ W^T (symmetric)
        idxT = mid_sb.tile([128, 128], F32, tag="idxT")        # [q,i] = idx[i*128+q]
        means = mid_sb.tile([K, C], F32, tag="means")
        meansT = mid_sb.tile([128, 2, K], F32, tag="meansT")
        Q_sb = mid_sb.tile([128, 2, K], F32, tag="Q_sb")
        aggT_sb = mid_sb.tile([128, 2, K], F32, tag="aggT_sb")
        B_sb = mid_sb.tile([K, K], F32, tag="B_sb")
        tmp64 = mid_sb.tile([K, K], F32, tag="tmp64")
        neg_g = mid_sb.tile([K, 1], F32, tag="neg_g")
        e_col = mid_sb.tile([K, 1], F32, tag="e_col")
        tableM = mid_sb.tile([K, C], F32, tag="tableM")
        tabh = mid_sb.tile([K, C], BF16, tag="tabh")

        # NOTE: a PSUM bank must never hold two OPEN accumulation groups —
        # the h=0 and h=1 halves of one po tile share a bank, so each h's
        # conv+gather group must fully close before the other h starts.
        def emit_group_conv(pos, base, h):
            xt_h = x_tiles[(base * P2_TILE) // PX_TILE]
            offs = [(base + i) * P2_TILE % PX_TILE for i in range(G)]
            sl = slice(h * P2_TILE, (h + 1) * P2_TILE)
            hs = slice(h * 128, (h + 1) * 128)
            for j in range(2):
                for i in range(G):
                    nc.tensor.matmul(
                        pos[i][:, sl], cwth_sb[:, j, hs],
                        xt_h[:, j, offs[i]:offs[i] + P2_TILE],
                        start=(j == 0), stop=False)

        def emit_group_gather(pos, base, h):
            sl = slice(h * P2_TILE, (h + 1) * P2_TILE)
            hs = slice(h * 128, (h + 1) * 128)
            for i in range(G):
                px0 = (base + i) * P2_TILE
                nc.tensor.matmul(
                    pos[i][:, sl], tabh[:, hs],
                    oh2_all[:, px0:px0 + P2_TILE],
                    start=False, stop=True)

        def emit_group_tail(pos, base, g):
            ot = p2_sb.tile([128, G, 2, P2_TILE], BF16, tag="ot",
                            name=f"ot{g}")
            for i in range(G):
                src = pos[i][:].rearrange("p (a b) -> p a b", a=2)
                if (base + i) % 2 == 0:
                    nc.scalar.copy(ot[:, i, :, :], src)
                else:
                    nc.vector.tensor_copy(ot[:, i, :, :], src)
            nc.sync.dma_start(
                out=out_v[g].rearrange("p (i h w) -> p i h w", i=G, h=2),
                in_=ot[:],
            )

        with (
            tc.tile_pool(name="psum_sums", bufs=1, space="PSUM") as pp_sums,
            tc.tile_pool(name="psum_mid", bufs=2, space="PSUM") as pp_mid,
            tc.tile_pool(name="psum_pre", bufs=G, space="PSUM") as pp_pre,
        ):
            psum_sums = pp_sums.tile([K, C], F32, tag="psum_sums")

            # Warm-up: make PE observe the POOL-produced identity before
            # the hot loop so transposes don't each carry a POOL wait.
            warm = pp_mid.tile([128, 128], F32, tag="pm")
            nc.tensor.transpose(warm[:], ident[:], ident[:])

            # M = W @ W^T: contract e; lhsT/rhs both W^T (e on partitions).
            for h in range(2):
                pm = pp_mid.tile([128, C], F32, tag="pm")
                for j in range(2):
                    nc.tensor.matmul(
                        pm[:], wt_sb[:, j, h * 128:(h + 1) * 128],
                        wt_sb[:, j, :], start=(j == 0), stop=(j == 1),
                    )
                nc.scalar.copy(M_sb[:, h, :], pm[:])

            # idxT: transpose idx_pm so column i = indices of pixel chunk i
            pi = pp_mid.tile([128, 128], F32, tag="pm")
            nc.tensor.transpose(pi[:], idx_pm[:], ident[:])
            nc.scalar.copy(idxT[:], pi[:])

            # ---- pass 1: segment sums over all pixels ----
            # x^T arrives pre-transposed from the host: no PE transposes,
            # just one oh-matmul per 128-px chunk, all accumulating into a
            # single PSUM tile (HAM stays warm on the 128-deep chain).
            with (
                tc.tile_pool(name="xt_pool", bufs=3) as xt_pool,
                tc.tile_pool(name="oh_pool", bufs=8) as oh_pool,
            ):
                first = True
                x_tiles = []
                for t in range(n_px_tiles):
                    u0 = t * chunks_per_px_tile
                    xtt = xt_pool.tile([128, chunks_per_px_tile, C], BF16,
                                       tag="xtt")
                    nc.sync.dma_start(
                        out=xtt[:],
                        in_=ins["xth"][t].rearrange(
                            "p (u c) -> p u c", c=C))
                    for q4 in range(chunks_per_px_tile // 4):
                        # one-hot for 4 chunks in one DVE op (per-op overhead
                        # dominates at [128,64]; broadcast APs batch it).
                        # fp8 one-hot (0/1 exact) to match the fp8 xT.
                        g4 = u0 + q4 * 4
                        oh4 = oh_pool.tile([128, 4, K], BF16, tag="oh4")
                        nc.vector.tensor_tensor(
                            out=oh4[:],
                            in0=idxT[:, g4:g4 + 4].unsqueeze(2)
                                .to_broadcast((128, 4, K)),
                            in1=iota_row[:].unsqueeze(1)
                                .to_broadcast((128, 4, K)),
                            op=ALU.is_equal)
                        for c4 in range(4):
                            gchunk = g4 + c4
                            nc.tensor.matmul(
                                psum_sums[:], oh4[:, c4, :],
                                xtt[:, q4 * 4 + c4, :],
                                start=first, stop=(gchunk == HW // 128 - 1))
                            first = False
                # queued after the xT stream: the index broadcast (needed by
                # the oh2 pre-gen ~10us later) and the original-layout x for
                # pass 2 (resident).
                nc.sync.dma_start(
                    out=idx_bc[:],
                    in_=ins["idxbf"].unsqueeze(0).to_broadcast((K, HW)),
                )
                for t in range(n_px_tiles):
                    xt_h = xres.tile([128, 2, PX_TILE], BF16, tag="xres")
                    x_tiles.append(xt_h)
                    for j in range(2):
                        nc.sync.dma_start(
                            out=xt_h[:, j, :],
                            in_=ins["xh"][j * 128:(j + 1) * 128,
                                          t * PX_TILE:(t + 1) * PX_TILE])

            # ---- middle: means -> adjacency -> table ----
            # Dummy N=64 matmuls keep the PE issue rate up through the
            # middle's serial DVE/Act chain so HAM doesn't drop to K=4/8
            # right before the pass-2 stream.
            def warm_pe(n=4):
                # dummy matmuls into the (already consumed) psum_sums region
                # keep the PE issue rate up through the middle's serial chain
                for _ in range(n):
                    nc.tensor.matmul(
                        psum_sums[0:K, 0:K], wt_sb[0:K, 0, 0:K],
                        wt_sb[0:K, 0, 0:K], start=True, stop=True)

            # prefill: group-0's h=0 conv matmuls (independent of the table)
            # keep the PE busy and warm through the middle's serial chain;
            # the h=0 gathers and all of h=1 follow once tabh is ready
            pre_pos = [pp_pre.tile([128, 2 * P2_TILE], F32, tag="po",
                                   name=f"pre{i}") for i in range(G)]
            emit_group_conv(pre_pos, 0, 0)
            nc.vector.tensor_scalar(
                out=means[:], in0=psum_sums[:], scalar1=recip_sb[:],
                scalar2=None, op0=ALU.mult,
            )
            # first pass-2 one-hots (DVE idles during the middle); 2048-px
            # ops amortize the per-op overhead
            OH2B = 2048
            for b2 in range(2):
                sl2 = slice(b2 * OH2B, (b2 + 1) * OH2B)
                nc.vector.tensor_scalar(
                    out=oh2_all[:, sl2], in0=idx_bc[:, sl2],
                    scalar1=iota_col[:], scalar2=None, op0=ALU.is_equal)
            warm_pe()

            # meansT (c on partitions)
            for h in range(2):
                pm = pp_mid.tile([128, K], F32, tag="pm")
                nc.tensor.transpose(
                    pm[:], means[:, h * 128:(h + 1) * 128], ident[0:K, 0:K],
                )
                nc.scalar.copy(meansT[:, h, :], pm[:])
            warm_pe()

            # ptm = means @ cwt, computed early: it's off the critical
            # adjacency chain and only needed at the final table combine
            ptm = pp_mid.tile([K, C], F32, tag="pm")
            for j in range(2):
                nc.tensor.matmul(
                    ptm[:], meansT[:, j, :], cwt_sb[:, j, :],
                    start=(j == 0), stop=(j == 1),
                )
            nc.scalar.copy(tableM[:], ptm[:])

            # Q = M @ means^T  (use symmetry of M for lhsT slicing)
            for h in range(2):
                pq = pp_mid.tile([128, K], F32, tag="pm")
                for dj in range(2):
                    nc.tensor.matmul(
                        pq[:], M_sb[:, dj, h * 128:(h + 1) * 128],
                        meansT[:, dj, :], start=(dj == 0), stop=(dj == 1),
                    )
                nc.scalar.copy(Q_sb[:, h, :], pq[:])
            warm_pe()

            # G = means @ Q  (64x64, symmetric)
            pg = pp_mid.tile([K, K], F32, tag="pm")
            for h in range(2):
                nc.tensor.matmul(
                    pg[:], meansT[:, h, :], Q_sb[:, h, :],
                    start=(h == 0), stop=(h == 1),
                )
            warm_pe()

            # -g = rowsum(G * (-I));  e_col = exp(-g);  B = exp(2G - g_i)
            nc.vector.scalar_tensor_tensor(
                out=tmp64[:], in0=pg[:], scalar=1.0, in1=negI[:],
                op0=ALU.mult, op1=ALU.mult, accum_out=neg_g[:],
            )
            warm_pe()
            nc.scalar.activation(e_col[:], neg_g[:], AF.Exp)
            nc.scalar.activation(B_sb[:], pg[:], AF.Exp, bias=neg_g[:], scale=2.0)
            warm_pe()

            # aggT_raw[c,i] = sum_j B[j,i] means[j,c]
            # (B[j,i] = exp(2G_ij - g_j) already carries e^{-g_j})
            for h in range(2):
                pa = pp_mid.tile([128, K], F32, tag="pm")
                nc.tensor.matmul(
                    pa[:], means[:, h * 128:(h + 1) * 128], B_sb[:],
                    start=True, stop=True,
                )
                nc.scalar.copy(aggT_sb[:, h, :], pa[:])
            warm_pe()
            # tabh[k, c_out] = bf16(e^{-g_k}*(aggT_raw^T@cwt)[k,:] - means@cwt)
            pt2 = pp_mid.tile([K, C], F32, tag="pm")
            for j in range(2):
                nc.tensor.matmul(
                    pt2[:], aggT_sb[:, j, :], cwt_sb[:, j, :],
                    start=(j == 0), stop=(j == 1),
                )
            warm_pe()
            nc.vector.scalar_tensor_tensor(
                out=tabh[:], in0=pt2[:], scalar=e_col[:], in1=tableM[:],
                op0=ALU.mult, op1=ALU.subtract,
            )
            warm_pe()
            # finish the prefilled group 0
            emit_group_gather(pre_pos, 0, 0)
            emit_group_conv(pre_pos, 0, 1)
            emit_group_gather(pre_pos, 0, 1)
            emit_group_tail(pre_pos, 0, 0)
            # rest of the pass-2 one-hots
            for b2 in range(2, HW // OH2B):
                sl2 = slice(b2 * OH2B, (b2 + 1) * OH2B)
                nc.vector.tensor_scalar(
                    out=oh2_all[:, sl2], in0=idx_bc[:, sl2],
                    scalar1=iota_col[:], scalar2=None, op0=ALU.is_equal)

        # ---- pass 2: out = conv_w @ x + table[index] ----
        # Grouped PE stream: G=4 tiles share each stationary weight load
        # (LDWEIGHTS is not hidden between distinct-weight matmuls), oh2
        # pre-generated so there are no per-tile engine round-trips, 7-deep
        # PSUM rotation, copies split across Act/DVE, one output DMA per
        # group with 4KB-contiguous lines (host descrambles the layout).
        with (
            tc.tile_pool(name="psum_p2", bufs=7, space="PSUM") as pp2,
            tc.tile_pool(name="psum_warm2", bufs=1, space="PSUM") as pp_w2,
        ):
            pwarm2 = pp_w2.tile([K, K], F32, tag="pwarm2")

            def kick_pe(n):
                # tiny matmuls raise the PE issue rate so the HAM clock gate
                # climbs back to (or stays at) K=8/8
                for _ in range(n):
                    nc.tensor.matmul(
                        pwarm2[:], wt_sb[0:K, 0, 0:K], wt_sb[0:K, 0, 0:K],
                        start=True, stop=True)

            kick_pe(6)
            for g in range(1, n_groups):
                if g % 4 == 2:
                    kick_pe(2)
                base = g * G
                pos = [pp2.tile([128, 2 * P2_TILE], F32, tag="po",
                                name=f"po{g}_{i}")
                       for i in range(G)]
                for h in range(2):
                    emit_group_conv(pos, base, h)
                    emit_group_gather(pos, base, h)
                emit_group_tail(pos, base, g)


def _ensure_ntff_hook():
    """Register the axon NTFF profiling hook if the image's antenv lacks it."""
    try:
        from antenv.axon_hooks import get_axon_ntff_profile_hook  # noqa: F401
        return
    except ImportError:
        pass
    import types

    import antenv

    mod = types.ModuleType("antenv.axon_hooks")
    _hook = [None]
    mod.set_axon_ntff_profile_hook = lambda h: _hook.__setitem__(0, h)
    mod.get_axon_ntff_profile_hook = lambda: _hook[0]
    sys.modules["antenv.axon_hooks"] = mod
    antenv.axon_hooks = mod
    try:
        from trn_agent_boot.trn_boot import _ntff_profile_via_ctypes

        so = "/opt/axon/libaxon_pjrt.so"
        if os.path.exists(so):
            mod.set_axon_ntff_profile_hook(_ntff_profile_via_ctypes(so))
    except Exception:
        pass


_NC_CACHE = None
LAST_RESULT = None


def _get_nc():
    global _NC_CACHE
    if _NC_CACHE is None:
        _NC_CACHE = build_nc()
    return _NC_CACHE


def kernel(x, index, W, conv_w):
    """Full inputs in, full output out. Shards batch across 8 NeuronCores."""
    global LAST_RESULT
    from concourse.bass_utils import run_bass_kernel_spmd

    import ml_dtypes

    x = np.asarray(x, dtype=np.float32).reshape(B, C, HW)
    idx_i = np.asarray(index).reshape(B, HW)
    idxf = idx_i.astype(np.float32)
    wt = np.ascontiguousarray(np.asarray(W, dtype=np.float32).T)
    cwt = np.ascontiguousarray(
        np.asarray(conv_w, dtype=np.float32).reshape(C, C).T
    )

    nc = _get_nc()
    # single bf16 plane for x / conv_w; segment-count reciprocals on host
    xh = x.astype(ml_dtypes.bfloat16)
    cwth = cwt.astype(ml_dtypes.bfloat16)
    idxbf = idxf.astype(ml_dtypes.bfloat16)
    counts = np.stack([np.bincount(idx_i[b], minlength=K) for b in range(B)])
    recip = (1.0 / np.maximum(counts, 1)).astype(np.float32)
    # x^T packed per px-tile into the SBUF layout [t, p, u, c] so each DMA
    # reads 8KB contiguous per partition
    n_pt = HW // 2048
    xtp = np.ascontiguousarray(
        np.transpose(
            np.swapaxes(xh, 1, 2).reshape(B, n_pt, 16, 128, C), (0, 1, 3, 2, 4)
        ).reshape(B, n_pt, 128, 16 * C)
    )
    in_maps = [
        {"xh": np.ascontiguousarray(xh[b]),
         "xth": xtp[b],
         "idxf": np.ascontiguousarray(idxf[b]),
         "idxbf": np.ascontiguousarray(idxbf[b]),
         "recip": recip[b],
         "wt": wt, "cwt": cwt, "cwth": cwth}
        for b in range(B)
    ]
    trace = bool(int(os.environ.get("KERNEL_TRACE", "0")))
    if trace:
        try:
            _ensure_ntff_hook()
            res = run_bass_kernel_spmd(
                nc, in_maps, core_ids=list(range(N_CORES)), trace=True,
            )
        except Exception as e:  # profiling must never break the answer path
            print(f"kernel: trace run failed ({e!r}); rerunning untraced")
            res = run_bass_kernel_spmd(
                nc, in_maps, core_ids=list(range(N_CORES)), trace=False,
            )
    else:
        res = run_bass_kernel_spmd(
            nc, in_maps, core_ids=list(range(N_CORES)), trace=False,
        )
    LAST_RESULT = res
    # descramble pass-2 group-tile layout: r[g, p, i, h, w] -> out[h*128+p, px]
    outs = []
    for b in range(B):
        r = np.asarray(res.results[b]["out"]).reshape(16, 128, 4, 2, 256)
        o = np.transpose(r, (3, 1, 0, 2, 4)).reshape(C, H, W_DIM)
        outs.append(o.astype(np.float32))
    return np.stack(outs)


# revision 55
# speedup vs baseline: 1.0244x; 1.0244x over previous
"""Trainium2 Bass kernel for the GNN message-passing module.

Per-sample pipeline (data-parallel: one batch element per NeuronCore):
  1. segment sums via one-hot matmul on PE (x transposed on-chip),
  2. small "middle" stage: means, M=W@W^T, Mahalanobis adjacency folded
     into a (K, C_out) table: table2T = adj-weighted conv'd means,
  3. out = conv_w @ x + table2T[index] via PE matmuls (the gather is a
     one-hot matmul accumulated into the same PSUM as the 1x1 conv).

Math notes:
  adj[i,j] = exp(-(m_j-m_i)^T M (m_j-m_i)) with zero diagonal, M=W W^T.
  Using G = means @ M @ means^T, g = diag(G):
    adj[i,j] = exp(2G_ij - g_i - g_j) - delta_ij
  agg = adj @ means  =>  out += conv_w @ agg[index]
  table2T[k,:] = e^{-g_k} * (aggT_raw^T @ conv_w^T)[k,:] - (means @ conv_w^T)[k,:]
  where aggT_raw[:,i] = sum_j B[j,i] * (e^{-g_j} means[j,:]),
        B[i,j] = exp(2G_ij - g_i).

Precision: single bf16 plane for x / conv_w / table / output with fp32
PSUM accumulation; middle stage fp32. Whole-pipeline numpy sim gives
rel err ~4e-3 against the fp32 reference (gate 2e-2). Segment counts
(and their reciprocals) are computed on the host and passed in.
"""

import os
import sys

import numpy as np


def _ensure_path():
    try:
        import concourse  # noqa: F401
    except ImportError:
        for p in ("/opt/trn_rl_repo", os.path.expanduser("~/.axon_site/_ro/trn_rl_repo")):
            if os.path.isdir(p) and p not in sys.path:
                sys.path.insert(0, p)


_ensure_path()
# persistent jax/XLA executable cache: makes repeat compiles of the same
# kernel cheap across processes (first compile of a variant is ~minutes).
os.environ.setdefault("JAX_COMPILATION_CACHE_DIR", "/tmp/jax_neff_cache")
os.environ.setdefault("JAX_PERSISTENT_CACHE_MIN_COMPILE_TIME_SECS", "10")

import concourse.bass as bass  # noqa: E402
import concourse.tile as tile  # noqa: E402
from concourse import bacc  # noqa: E402
from concourse import mybir  # noqa: E402
from concourse.masks import make_identity  # noqa: E402

F32 = mybir.dt.float32

# --- workaround: this walrus build rejects instructions carrying >2 sem
# waits ("Too many sync wait commands" in setupSyncWait). TileContext's exit
# drain accumulates one wait per outstanding processor (DMA queues etc.), so
# split them across NOPs emitted just before the drain. Semaphores are
# monotonic, so waiting earlier on the same conditions is equivalent.
_MAX_WAITS = 1
_drain_patched = False


def _patch_tile_drain():
    global _drain_patched
    if _drain_patched:
        return
    _drain_patched = True
    from concourse.vector_clock import ScopedClock

    orig = tile.TileContext._drain_and_barrier

    def patched(self, tick_clock, wait_clock):
        nc = self.nc
        probe = nc.sync.nop()
        wait_clock.add_sem_waits(
            probe.ins, ScopedClock({None: tick_clock.global_clock})
        )
        waits = list(probe.ins.sync_info.on_wait or [])
        chunks = [waits[i:i + _MAX_WAITS] for i in range(0, len(waits), _MAX_WAITS)]
        probe.ins.sync_info.on_wait = chunks[0] if chunks else []
        for chunk in chunks[1:]:
            nop = nc.sync.nop()
            nop.ins.sync_info = mybir.SyncInfo(on_wait=chunk, on_update=[])
        orig(self, tick_clock, wait_clock)
        _trim_redundant_waits(nc)

    tile.TileContext._drain_and_barrier = patched


def _trim_redundant_waits(nc):
    """Transitive wait reduction. Tile's add_semaphores is per-instruction
    minimal but not transitively minimal across processors: an instruction
    often carries waits already implied by (a) an earlier wait on the same
    engine, or (b) the closure of another wait it carries (the producer's own
    waits + in-order retirement on the producer's engine). This walrus build
    rejects instructions with >2 sync waits, so prune implied waits.

    Soundness assumptions: sem updates fire at instruction retirement;
    retirement is in-order per compute engine and per DMA queue sem (one sem
    per queue, FIFO); a kept wait on sem S>=v implies the v-reaching update's
    instruction retired, hence its dispatch-time holds and (non-DMA) all
    earlier same-engine updates.
    """
    import bisect

    for blk in nc.m.functions[0].blocks:
        insts = list(blk.instructions)
        n = len(insts)
        # sems that are ever decremented/reset are not monotonic; leave all
        # waits on them untouched and exclude them from closures (barrier
        # gather/release sems, end-of-kernel sem clears).
        nonmono = set()
        for ins in insts:
            si = ins.sync_info
            if si and si.on_update:
                for u in si.on_update:
                    if u.update_mode != "sem-inc":
                        nonmono.add(u.id)
            try:
                if ins.is_reset_sema:
                    lo = ins.reset_range_start
                    hi = ins.reset_range_stop
                    if lo is not None and hi is not None:
                        nonmono.update(range(lo, hi + 1))
            except Exception:
                pass
        upd = {}
        cum = {}
        own_cum_after = [None] * n
        eng_of = [str(i.engine) for i in insts]
        is_dma = [type(i).__name__ == "InstDMACopy" for i in insts]
        for idx, ins in enumerate(insts):
            si = ins.sync_info
            d = {}
            if si and si.on_update:
                for u in si.on_update:
                    if (u.update_mode != "sem-inc" or not u.update_value
                            or u.id in nonmono):
                        continue
                    c = cum.get(u.id, 0) + u.update_value
                    cum[u.id] = c
                    upd.setdefault(u.id, []).append((c, idx))
                    d[u.id] = c
            own_cum_after[idx] = d
        eng_cum_after = [None] * n
        run = {}
        for idx in range(n):
            e = eng_of[idx]
            m = dict(run.get(e, {}))
            if not is_dma[idx]:
                for s, c in own_cum_after[idx].items():
                    m[s] = c
            run[e] = m
            eng_cum_after[idx] = m

        def updater_idx(sem, v):
            lst = upd.get(sem)
            if not lst:
                return None
            pos = bisect.bisect_left(lst, (v, -1))
            if pos == len(lst):
                return None
            return lst[pos][1]

        holds_at = [None] * n
        last_eng = {}
        memo = {}

        def completion_holds(uidx):
            if uidx in memo:
                return memo[uidx]
            h = dict(holds_at[uidx] or {})
            src_cum = own_cum_after[uidx] if is_dma[uidx] else eng_cum_after[uidx]
            for s, c in src_cum.items():
                if h.get(s, 0) < c:
                    h[s] = c
            memo[uidx] = h
            return h

        n_dropped = 0
        for idx, ins in enumerate(insts):
            e = eng_of[idx]
            base = dict(holds_at[last_eng[e]]) if e in last_eng else {}
            si = ins.sync_info
            if si and si.on_wait:
                kept = []
                for w in si.on_wait:
                    if w.wait_mode != "sem-ge-imm" or w.id in nonmono:
                        kept.append(w)
                        continue
                    if base.get(w.id, 0) >= w.wait_value:
                        n_dropped += 1
                        continue
                    kept.append(w)
                    ui = updater_idx(w.id, w.wait_value)
                    if ui is not None and ui < idx:
                        for s, v in completion_holds(ui).items():
                            if base.get(s, 0) < v:
                                base[s] = v
                    if base.get(w.id, 0) < w.wait_value:
                        base[w.id] = w.wait_value
                if len(kept) != len(si.on_wait):
                    si.on_wait = kept
            holds_at[idx] = base
            last_eng[e] = idx
_compile_patched = False


def _patch_compile_bir():
    """This walrus build accepts at most ONE sync wait per instruction in
    several encodings (S3_LW matmuls, CTRL NoOp/Drain). Tile legitimately
    emits 2 waits on some instructions, so rewrite the serialized BIR just
    before walrus: keep one wait on the instruction and hoist the rest onto
    same-engine NoOps inserted immediately before it (same dispatch point,
    so semantics are unchanged)."""
    global _compile_patched
    if _compile_patched:
        return
    _compile_patched = True
    import orjson

    from concourse import bass2jax, bass_utils

    orig = bass_utils.compile_bir_kernel

    def _split_waits(bir_json: bytes) -> bytes:
        d = orjson.loads(bir_json)
        changed = False
        for fn in d.get("functions", []):
            for blk in fn.get("blocks", []):
                insts = blk.get("instructions", [])
                out = []
                for inst in insts:
                    si = inst.get("sync_info") or {}
                    ow = si.get("on_wait") or []
                    if len(ow) > 1:
                        changed = True
                        for k, w in enumerate(ow[:-1]):
                            out.append({
                                "debug": inst.get("debug", 0),
                                "engine": inst["engine"],
                                "ins": [],
                                "name": f"{inst['name']}-w{k}",
                                "opcode": "NoOp",
                                "outs": [],
                                "sync_info": {"on_update": [],
                                              "on_wait": [w]},
                            })
                        si["on_wait"] = [ow[-1]]
                    out.append(inst)
                blk["instructions"] = out
        return orjson.dumps(d) if changed else bir_json

    def wrapper(bir_json, tmpdir, neff_name="file.neff"):
        return orig(_split_waits(bir_json), tmpdir, neff_name=neff_name)

    bass_utils.compile_bir_kernel = wrapper
    bass2jax.compile_bir_kernel = wrapper


AF = mybir.ActivationFunctionType
ALU = mybir.AluOpType

B, C, K, H, W_DIM = 8, 256, 64, 128, 128
HW = H * W_DIM  # 16384 pixels per sample
N_CORES = 8

PX_TILE = 2048        # pass-1 x DMA tile (pixels)
P2_TILE = 256         # pass-2 pixel tile (N=256 keeps the HAM clock-gate at
                      # K=8/8: N=512 streams issue too slowly and get stuck
                      # at half clock — measured 500ns/MM vs ~150ns)


def build_nc():
    _patch_tile_drain()
    _patch_compile_bir()
    # Bacc (not raw Bass): its compile() pass auto-inserts the GPSIMD
    # library reloads that ap_gather needs, in correct program order.
    nc = bacc.Bacc("TRN2", target_bir_lowering=False, debug=False)
    BF16 = mybir.dt.bfloat16
    idx_d = nc.dram_tensor("idxf", (HW,), F32, kind="ExternalInput")
    wt_d = nc.dram_tensor("wt", (C, C), F32, kind="ExternalInput")      # W^T
    # output in pass-2 group-tile order (host descrambles): [g, p, i*512+h*256+w]
    out_d = nc.dram_tensor("out", (HW // (4 * P2_TILE), 128, 4 * 2 * P2_TILE),
                           BF16, kind="ExternalOutput")
    ins = dict(
        xh=nc.dram_tensor("xh", (C, HW), BF16, kind="ExternalInput").ap(),
        # x^T pre-packed on host into per-px-tile SBUF layout: [t, p, u*256+c]
        # so each DMA line is 8KB contiguous per partition
        xth=nc.dram_tensor("xth", (HW // PX_TILE, 128,
                                   (PX_TILE // 128) * C), BF16,
                           kind="ExternalInput").ap(),
        cwth=nc.dram_tensor("cwth", (C, C), BF16, kind="ExternalInput").ap(),
        cwt=nc.dram_tensor("cwt", (C, C), F32, kind="ExternalInput").ap(),
        idxbf=nc.dram_tensor("idxbf", (HW,), BF16, kind="ExternalInput").ap(),
        recip=nc.dram_tensor("recip", (K,), F32, kind="ExternalInput").ap(),
    )

    with tile.TileContext(nc) as tc:
        _body(tc, ins, idx_d.ap(), wt_d.ap(), out_d.ap())
    nc.compile()
    return nc


def _body(tc, ins, idx_v, wt_v, out_v):
    nc = tc.nc
    BF16 = mybir.dt.bfloat16
    n_px_tiles = HW // PX_TILE              # 8
    chunks_per_px_tile = PX_TILE // 128     # 16
    n_p2_tiles = HW // P2_TILE              # 32
    cwt_v = ins["cwt"]

    with (
        tc.tile_pool(name="consts", bufs=1) as consts,
        tc.tile_pool(name="xres", bufs=n_px_tiles) as xres,
        tc.tile_pool(name="mid_sb", bufs=1) as mid_sb,
    ):
        # ---- constants / parameter loads ----
        ident = consts.tile([128, 128], F32, tag="ident")
        make_identity(nc, ident[:])

        iota_row = consts.tile([128, K], F32, tag="iota_row")  # [p,k] = k
        iota_row_i = consts.tile([128, K], mybir.dt.int32, tag="iota_row_i")
        nc.gpsimd.iota(iota_row_i[:], pattern=[[1, K]], base=0, channel_multiplier=0)
        nc.vector.tensor_copy(iota_row[:], iota_row_i[:])

        negI = consts.tile([K, K], F32, tag="negI")            # -identity(64)
        nc.gpsimd.memset(negI[:], 0.0)
        nc.gpsimd.affine_select(
            out=negI[:], in_=negI[:], compare_op=ALU.not_equal,
            fill=-1.0, base=0, pattern=[[-1, K]], channel_multiplier=1,
        )

        wt_sb = consts.tile([128, 2, C], F32, tag="wt_sb")     # [e, j, c] = W^T[j*128+e, c]
        nc.sync.dma_start(out=wt_sb[:], in_=wt_v.rearrange("(j p) c -> p j c", p=128))
        cwt_sb = consts.tile([128, 2, C], F32, tag="cwt_sb")   # [ci, j, co]
        nc.sync.dma_start(out=cwt_sb[:], in_=cwt_v.rearrange("(j p) c -> p j c", p=128))
        cwth_sb = consts.tile([128, 2, C], BF16, tag="cwth_sb")
        nc.sync.dma_start(
            out=cwth_sb[:],
            in_=ins["cwth"].rearrange("(j p) c -> p j c", p=128))

        idx_pm = consts.tile([128, 128], F32, tag="idx_pm")    # [p,f] = idx[p*128+f]
        nc.sync.dma_start(out=idx_pm[:], in_=idx_v.rearrange("(p f) -> p f", p=128))

        recip_sb = consts.tile([K, 1], F32, tag="recip_sb")    # 1/count (host)
        nc.sync.dma_start(out=recip_sb[:], in_=ins["recip"].rearrange("(p o) -> p o", o=1))

        iota_col = consts.tile([K, 1], F32, tag="iota_col")    # [k,0] = k
        iota_col_i = consts.tile([K, 1], mybir.dt.int32, tag="iota_col_i")
        nc.gpsimd.iota(iota_col_i[:], pattern=[[1, 1]], base=0,
                       channel_multiplier=1)
        nc.vector.tensor_copy(iota_col[:], iota_col_i[:])
        # index values broadcast to K partitions (bf16: 0..63 exact); the
        # DMA is emitted mid-way through the pass-1 xT stream (needed ~20us
        # in, must not delay the first xT tiles).
        idx_bc = consts.tile([K, HW], BF16, tag="idx_bc")
        # pass-2 one-hot [k, px], pre-generated during pass1/middle so the
        # pass-2 PE stream never round-trips through DVE (HAM warmth).
        oh2_all = consts.tile([K, HW], BF16, tag="oh2_all")

        M_sb = mid_sb.tile([128, 2, C], F32, tag="M_sb")       # M = W @ W^T (symmetric)
        idxT = mid_sb.tile([128, 128], F32, tag="idxT")        # [q,i] = idx[i*128+q]
        means = mid_sb.tile([K, C], F32, tag="means")
        meansT = mid_sb.tile([128, 2, K], F32, tag="meansT")
        Q_sb = mid_sb.tile([128, 2, K], F32, tag="Q_sb")
        aggT_sb = mid_sb.tile([128, 2, K], F32, tag="aggT_sb")
        B_sb = mid_sb.tile([K, K], F32, tag="B_sb")
        tmp64 = mid_sb.tile([K, K], F32, tag="tmp64")
        neg_g = mid_sb.tile([K, 1], F32, tag="neg_g")
        e_col = mid_sb.tile([K, 1], F32, tag="e_col")
        tableM = mid_sb.tile([K, C], F32, tag="tableM")
        tabh = mid_sb.tile([K, C], BF16, tag="tabh")

        with (
            tc.tile_pool(name="psum_sums", bufs=1, space="PSUM") as pp_sums,
            tc.tile_pool(name="psum_mid", bufs=2, space="PSUM") as pp_mid,
            tc.tile_pool(name="psum_warm", bufs=1, space="PSUM") as pp_warm,
        ):
            psum_sums = pp_sums.tile([K, C], F32, tag="psum_sums")

            # Warm-up: make PE observe the POOL-produced identity before
            # the hot loop so transposes don't each carry a POOL wait.
            warm = pp_mid.tile([128, 128], F32, tag="pm")
            nc.tensor.transpose(warm[:], ident[:], ident[:])

            # M = W @ W^T: contract e; lhsT/rhs both W^T (e on partitions).
            for h in range(2):
                pm = pp_mid.tile([128, C], F32, tag="pm")
                for j in range(2):
                    nc.tensor.matmul(
                        pm[:], wt_sb[:, j, h * 128:(h + 1) * 128],
                        wt_sb[:, j, :], start=(j == 0), stop=(j == 1),
                    )
                nc.scalar.copy(M_sb[:, h, :], pm[:])

            # idxT: transpose idx_pm so column i = indices of pixel chunk i
            pi = pp_mid.tile([128, 128], F32, tag="pm")
            nc.tensor.transpose(pi[:], idx_pm[:], ident[:])
            nc.scalar.copy(idxT[:], pi[:])

            # ---- pass 1: segment sums over all pixels ----
            # x^T arrives pre-transposed from the host: no PE transposes,
            # just one oh-matmul per 128-px chunk, all accumulating into a
            # single PSUM tile (HAM stays warm on the 128-deep chain).
            with (
                tc.tile_pool(name="xt_pool", bufs=3) as xt_pool,
                tc.tile_pool(name="oh_pool", bufs=8) as oh_pool,
            ):
                first = True
                x_tiles = []
                for t in range(n_px_tiles):
                    u0 = t * chunks_per_px_tile
                    xtt = xt_pool.tile([128, chunks_per_px_tile, C], BF16,
                                       tag="xtt")
                    nc.sync.dma_start(
                        out=xtt[:],
                        in_=ins["xth"][t].rearrange(
                            "p (u c) -> p u c", c=C))
                    for q4 in range(chunks_per_px_tile // 4):
                        # one-hot for 4 chunks in one DVE op (per-op overhead
                        # dominates at [128,64]; broadcast APs batch it).
                        # fp8 one-hot (0/1 exact) to match the fp8 xT.
                        g4 = u0 + q4 * 4
                        oh4 = oh_pool.tile([128, 4, K], BF16, tag="oh4")
                        nc.vector.tensor_tensor(
                            out=oh4[:],
                            in0=idxT[:, g4:g4 + 4].unsqueeze(2)
                                .to_broadcast((128, 4, K)),
                            in1=iota_row[:].unsqueeze(1)
                                .to_broadcast((128, 4, K)),
                            op=ALU.is_equal)
                        for c4 in range(4):
                            gchunk = g4 + c4
                            nc.tensor.matmul(
                                psum_sums[:], oh4[:, c4, :],
                                xtt[:, q4 * 4 + c4, :],
                                start=first, stop=(gchunk == HW // 128 - 1))
                            first = False
                # queued after the xT stream: the index broadcast (needed by
                # the oh2 pre-gen ~10us later) and the original-layout x for
                # pass 2 (resident).
                nc.sync.dma_start(
                    out=idx_bc[:],
                    in_=ins["idxbf"].unsqueeze(0).to_broadcast((K, HW)),
                )
                for t in range(n_px_tiles):
                    xt_h = xres.tile([128, 2, PX_TILE], BF16, tag="xres")
                    x_tiles.append(xt_h)
                    for j in range(2):
                        nc.sync.dma_start(
                            out=xt_h[:, j, :],
                            in_=ins["xh"][j * 128:(j + 1) * 128,
                                          t * PX_TILE:(t + 1) * PX_TILE])

            # ---- middle: means -> adjacency -> table ----
            # Dummy N=64 matmuls keep the PE issue rate up through the
            # middle's serial DVE/Act chain so HAM doesn't drop to K=4/8
            # right before the pass-2 stream.
            pwarm = pp_warm.tile([K, K], F32, tag="pwarm")

            def warm_pe(n=4):
                for _ in range(n):
                    nc.tensor.matmul(
                        pwarm[:], wt_sb[0:K, 0, 0:K], wt_sb[0:K, 0, 0:K],
                        start=True, stop=True)

            warm_pe(6)
            nc.vector.tensor_scalar(
                out=means[:], in0=psum_sums[:], scalar1=recip_sb[:],
                scalar2=None, op0=ALU.mult,
            )
            # first pass-2 one-hots (DVE idles during the middle); 2048-px
            # ops amortize the per-op overhead
            OH2B = 2048
            for b2 in range(2):
                sl2 = slice(b2 * OH2B, (b2 + 1) * OH2B)
                nc.vector.tensor_scalar(
                    out=oh2_all[:, sl2], in0=idx_bc[:, sl2],
                    scalar1=iota_col[:], scalar2=None, op0=ALU.is_equal)
            warm_pe()

            # meansT (c on partitions)
            for h in range(2):
                pm = pp_mid.tile([128, K], F32, tag="pm")
                nc.tensor.transpose(
                    pm[:], means[:, h * 128:(h + 1) * 128], ident[0:K, 0:K],
                )
                nc.scalar.copy(meansT[:, h, :], pm[:])
            warm_pe()

            # ptm = means @ cwt, computed early: it's off the critical
            # adjacency chain and only needed at the final table combine
            ptm = pp_mid.tile([K, C], F32, tag="ptm")
            for j in range(2):
                nc.tensor.matmul(
                    ptm[:], meansT[:, j, :], cwt_sb[:, j, :],
                    start=(j == 0), stop=(j == 1),
                )
            nc.scalar.copy(tableM[:], ptm[:])

            # Q = M @ means^T  (use symmetry of M for lhsT slicing)
            for h in range(2):
                pq = pp_mid.tile([128, K], F32, tag="pm")
                for dj in range(2):
                    nc.tensor.matmul(
                        pq[:], M_sb[:, dj, h * 128:(h + 1) * 128],
                        meansT[:, dj, :], start=(dj == 0), stop=(dj == 1),
                    )
                nc.scalar.copy(Q_sb[:, h, :], pq[:])
            warm_pe()

            # G = means @ Q  (64x64, symmetric)
            pg = pp_mid.tile([K, K], F32, tag="pm")
            for h in range(2):
                nc.tensor.matmul(
                    pg[:], meansT[:, h, :], Q_sb[:, h, :],
                    start=(h == 0), stop=(h == 1),
                )
            warm_pe()

            # -g = rowsum(G * (-I));  e_col = exp(-g);  B = exp(2G - g_i)
            nc.vector.scalar_tensor_tensor(
                out=tmp64[:], in0=pg[:], scalar=1.0, in1=negI[:],
                op0=ALU.mult, op1=ALU.mult, accum_out=neg_g[:],
            )
            warm_pe()
            nc.scalar.activation(e_col[:], neg_g[:], AF.Exp)
            nc.scalar.activation(B_sb[:], pg[:], AF.Exp, bias=neg_g[:], scale=2.0)
            warm_pe()

            # aggT_raw[c,i] = sum_j B[j,i] means[j,c]
            # (B[j,i] = exp(2G_ij - g_j) already carries e^{-g_j})
            for h in range(2):
                pa = pp_mid.tile([128, K], F32, tag="pm")
                nc.tensor.matmul(
                    pa[:], means[:, h * 128:(h + 1) * 128], B_sb[:],
                    start=True, stop=True,
                )
                nc.scalar.copy(aggT_sb[:, h, :], pa[:])
            warm_pe()
            # tabh[k, c_out] = bf16(e^{-g_k}*(aggT_raw^T@cwt)[k,:] - means@cwt)
            pt2 = pp_mid.tile([K, C], F32, tag="pm")
            for j in range(2):
                nc.tensor.matmul(
                    pt2[:], aggT_sb[:, j, :], cwt_sb[:, j, :],
                    start=(j == 0), stop=(j == 1),
                )
            warm_pe()
            nc.vector.scalar_tensor_tensor(
                out=tabh[:], in0=pt2[:], scalar=e_col[:], in1=tableM[:],
                op0=ALU.mult, op1=ALU.subtract,
            )
            warm_pe()
            # rest of the pass-2 one-hots
            for b2 in range(2, HW // OH2B):
                sl2 = slice(b2 * OH2B, (b2 + 1) * OH2B)
                nc.vector.tensor_scalar(
                    out=oh2_all[:, sl2], in0=idx_bc[:, sl2],
                    scalar1=iota_col[:], scalar2=None, op0=ALU.is_equal)

        # ---- pass 2: out = conv_w @ x + table[index] ----
        # Grouped PE stream: G=4 tiles share each stationary weight load
        # (LDWEIGHTS is not hidden between distinct-weight matmuls), oh2
        # pre-generated so there are no per-tile engine round-trips, 7-deep
        # PSUM rotation, copies split across Act/DVE, one output DMA per
        # group with 4KB-contiguous lines (host descrambles the layout).
        G = 4
        n_groups = n_p2_tiles // G
        out_a = out_v
        with (
            tc.tile_pool(name="psum_p2", bufs=7, space="PSUM") as pp2,
            tc.tile_pool(name="psum_warm2", bufs=1, space="PSUM") as pp_w2,
            tc.tile_pool(name="p2_sb", bufs=3) as p2_sb,
        ):
            pwarm2 = pp_w2.tile([K, K], F32, tag="pwarm2")

            def kick_pe(n):
                # tiny matmuls raise the PE issue rate so the HAM clock gate
                # climbs back to (or stays at) K=8/8
                for _ in range(n):
                    nc.tensor.matmul(
                        pwarm2[:], wt_sb[0:K, 0, 0:K], wt_sb[0:K, 0, 0:K],
                        start=True, stop=True)

            kick_pe(8)
            for g in range(n_groups):
                if g % 4 == 2:
                    kick_pe(2)
                base = g * G
                pt_ = (base * P2_TILE) // PX_TILE
                xt_h = x_tiles[pt_]
                offs = [(base + i) * P2_TILE % PX_TILE for i in range(G)]
                pos = [pp2.tile([128, 2 * P2_TILE], F32, tag="po",
                                name=f"po{g}_{i}")
                       for i in range(G)]
                ot = p2_sb.tile([128, G, 2, P2_TILE], BF16, tag="ot")
                for h in range(2):
                    sl = slice(h * P2_TILE, (h + 1) * P2_TILE)
                    hs = slice(h * 128, (h + 1) * 128)
                    for j in range(2):
                        for i in range(G):
                            nc.tensor.matmul(
                                pos[i][:, sl], cwth_sb[:, j, hs],
                                xt_h[:, j, offs[i]:offs[i] + P2_TILE],
                                start=(j == 0), stop=False)
                    for i in range(G):
                        px0 = (base + i) * P2_TILE
                        nc.tensor.matmul(
                            pos[i][:, sl], tabh[:, hs],
                            oh2_all[:, px0:px0 + P2_TILE],
                            start=False, stop=True)
                for i in range(G):
                    src = pos[i][:].rearrange("p (a b) -> p a b", a=2)
                    if (base + i) % 2 == 0:
                        nc.scalar.copy(ot[:, i, :, :], src)
                    else:
                        nc.vector.tensor_copy(ot[:, i, :, :], src)
                nc.sync.dma_start(
                    out=out_a[g].rearrange("p (i h w) -> p i h w",
                                           i=G, h=2),
                    in_=ot[:],
                )


def _ensure_ntff_hook():
    """Register the axon NTFF profiling hook if the image's antenv lacks it."""
    try:
        from antenv.axon_hooks import get_axon_ntff_profile_hook  # noqa: F401
        return
    except ImportError:
        pass
    import types

    import antenv

    mod = types.ModuleType("antenv.axon_hooks")
    _hook = [None]
    mod.set_axon_ntff_profile_hook = lambda h: _hook.__setitem__(0, h)
    mod.get_axon_ntff_profile_hook = lambda: _hook[0]
    sys.modules["antenv.axon_hooks"] = mod
    antenv.axon_hooks = mod
    try:
        from trn_agent_boot.trn_boot import _ntff_profile_via_ctypes

        so = "/opt/axon/libaxon_pjrt.so"
        if os.path.exists(so):
            mod.set_axon_ntff_profile_hook(_ntff_profile_via_ctypes(so))
    except Exception:
        pass


_NC_CACHE = None
LAST_RESULT = None


def _get_nc():
    global _NC_CACHE
    if _NC_CACHE is None:
        _NC_CACHE = build_nc()
    return _NC_CACHE


def kernel(x, index, W, conv_w):
    """Full inputs in, full output out. Shards batch across 8 NeuronCores."""
    global LAST_RESULT
    from concourse.bass_utils import run_bass_kernel_spmd

    import ml_dtypes

    x = np.asarray(x, dtype=np.float32).reshape(B, C, HW)
    idx_i = np.asarray(index).reshape(B, HW)
    idxf = idx_i.astype(np.float32)
    wt = np.ascontiguousarray(np.asarray(W, dtype=np.float32).T)
    cwt = np.ascontiguousarray(
        np.asarray(conv_w, dtype=np.float32).reshape(C, C).T
    )

    nc = _get_nc()
    # single bf16 plane for x / conv_w; segment-count reciprocals on host
    xh = x.astype(ml_dtypes.bfloat16)
    cwth = cwt.astype(ml_dtypes.bfloat16)
    idxbf = idxf.astype(ml_dtypes.bfloat16)
    counts = np.stack([np.bincount(idx_i[b], minlength=K) for b in range(B)])
    recip = (1.0 / np.maximum(counts, 1)).astype(np.float32)
    # x^T packed per px-tile into the SBUF layout [t, p, u, c] so each DMA
    # reads 8KB contiguous per partition
    n_pt = HW // 2048
    xtp = np.ascontiguousarray(
        np.transpose(
            np.swapaxes(xh, 1, 2).reshape(B, n_pt, 16, 128, C), (0, 1, 3, 2, 4)
        ).reshape(B, n_pt, 128, 16 * C)
    )
    in_maps = [
        {"xh": np.ascontiguousarray(xh[b]),
         "xth": xtp[b],
         "idxf": np.ascontiguousarray(idxf[b]),
         "idxbf": np.ascontiguousarray(idxbf[b]),
         "recip": recip[b],
         "wt": wt, "cwt": cwt, "cwth": cwth}
        for b in range(B)
    ]
    trace = bool(int(os.environ.get("KERNEL_TRACE", "0")))
    if trace:
        try:
            _ensure_ntff_hook()
            res = run_bass_kernel_spmd(
                nc, in_maps, core_ids=list(range(N_CORES)), trace=True,
            )
        except Exception as e:  # profiling must never break the answer path
            print(f"kernel: trace run failed ({e!r}); rerunning untraced")
            res = run_bass_kernel_spmd(
                nc, in_maps, core_ids=list(range(N_CORES)), trace=False,
            )
    else:
        res = run_bass_kernel_spmd(
            nc, in_maps, core_ids=list(range(N_CORES)), trace=False,
        )
    LAST_RESULT = res
    # descramble pass-2 group-tile layout: r[g, p, i, h, w] -> out[h*128+p, px]
    outs = []
    for b in range(B):
        r = np.asarray(res.results[b]["out"]).reshape(16, 128, 4, 2, 256)
        o = np.transpose(r, (3, 1, 0, 2, 4)).reshape(C, H, W_DIM)
        outs.append(o.astype(np.float32))
    return np.stack(outs)
